# revision 1
# baseline (speedup 1.0000x reference)
"""Trainium2 Bass kernel for nn_ExactDivergenceModel (retrieval_knn).

Math (per batch b):
  XX[i,j] = ||X[i]-X[j]||, YX[i,j] = ||X[i]-Y[j]||
  out[b]  = (1/N) sum_i ( log min_{j!=i} XX[i,j] - log min_j YX[i,j] )
which only needs per-row minima of the squared-distance matrices:
  d2_XX[i,j] = x2[j] - 2<X_i,X_j>  (+ x2[i] added on host)
  d2_YX[i,j] = y2[j] - 2<X_i,Y_j>  (+ x2[i] added on host)

Device strategy (1 batch per NeuronCore, 8 cores):
  - Augmented matmul, K = D+2 = 66: lhsT = [-2*X^T; 1; 1], rhs = [Y^T; y2_hi; y2_lo]
    so PSUM directly holds y2[j] - 2<X_i, Y_j>. fp32r matmuls (1 cyc/row).
  - Diagonal of XX masked by accumulating BIG*I via a second matmul
    (lhsT = rhs = sqrt(BIG)*I_128, start=False) - PE-only, no vector cost.
  - Row minima via one VectorE tensor_reduce(min) per full PSUM row [128, 4096].
  - Host adds x2[i], applies eps clamp + log + mean in float64.
  - Default path is a raw-bacc build (_build_raw): semaphores ride on the
    compute instructions, eliminating the Tile scheduler's 128 standalone
    per-iteration EventSemaphore instructions (this backend's cost is
    dominated by per-instruction overhead). Tile build kept as fallback.
"""
import sys, time
sys.path.insert(0, '/opt/trn_rl_repo')

import numpy as np
import ml_dtypes

import concourse.bass as bass
import concourse.tile as tile
from concourse import bacc, mybir
from concourse.bass_utils import run_bass_kernel_spmd

B, N, D = 8, 4096, 64
P = 128                 # partitions / i-block size
NB = N // P             # 32 i-blocks
K = D + 2               # contraction with the two norm rows
HALF = 2048             # psum half-row width
EPS = 1e-12
SQRT_BIG = 32768.0      # BIG = 2^30 on the XX diagonal

_cache = {}

MM_DTYPE = "float32r"   # "float32r" | "float16" | "bfloat16" | "bf16x2"
MM_W = 512              # matmul free-dim width (chunk)
RED_W = 4096            # reduce width (psum tile width)


def _build(repeat=1, mmdt_name=None, mm_w=None, red_w=None, skip_reduce=False,
           skip_mm=False):
    mmdt_name = mmdt_name or MM_DTYPE
    mm_w = mm_w or MM_W
    red_w = red_w or RED_W
    assert red_w % mm_w == 0 and N % red_w == 0
    n_tiles = N // red_w            # psum tiles per (block, matrix)
    n_ch = red_w // mm_w            # matmuls per psum tile
    psum_bufs = 1 if red_w == 4096 else 2
    nc = bacc.Bacc(None, target_bir_lowering=False)
    f32 = mybir.dt.float32
    mmdt = None if mmdt_name == "bf16x2" else getattr(mybir.dt, mmdt_name)

    bf16x2 = mmdt_name == "bf16x2"
    if bf16x2:
        K1, K2 = D + 2, 2 * D
        L_d = nc.dram_tensor("L", [K1, N], f32, kind="ExternalInput")     # [-2Xhi^T; 1]
        RX_d = nc.dram_tensor("RX", [K1, N], f32, kind="ExternalInput")   # [Xhi^T; x2h]
        RY_d = nc.dram_tensor("RY", [K1, N], f32, kind="ExternalInput")
        L2_d = nc.dram_tensor("L2", [K2, N], f32, kind="ExternalInput")   # [-2Xhi^T; -2Xlo^T; 1]
        RX2_d = nc.dram_tensor("RX2", [K2, N], f32, kind="ExternalInput") # [Xlo^T; Xhi^T; x2lo]
        RY2_d = nc.dram_tensor("RY2", [K2, N], f32, kind="ExternalInput")
        mmdt = mybir.dt.bfloat16
    else:
        RX_d = nc.dram_tensor("RX", [K, N], f32, kind="ExternalInput")
        RY_d = nc.dram_tensor("RY", [K, N], f32, kind="ExternalInput")
    EYE_d = nc.dram_tensor("EYE", [P, P], f32, kind="ExternalInput")
    MX_d = nc.dram_tensor("MX", [P, n_tiles * NB], f32, kind="ExternalOutput")
    MY_d = nc.dram_tensor("MY", [P, n_tiles * NB], f32, kind="ExternalOutput")

    with tile.TileContext(nc) as tc:
        with tc.tile_pool(name="const", bufs=1) as const, \
             tc.tile_pool(name="psum", bufs=psum_bufs, space="PSUM") as psum, \
             tc.tile_pool(name="outs", bufs=1) as outs:
            KA = (D + 2) if bf16x2 else K
            Lf = const.tile([KA, N], f32)
            RXf = const.tile([KA, N], f32)
            RYf = const.tile([KA, N], f32)
            EYEf = const.tile([P, P], f32)
            if bf16x2:
                nc.sync.dma_start(out=Lf, in_=L_d[:])
            else:
                # L = [-2*X^T; 1; 1] derived from RX = [X^T; x2h; x2l]
                nc.vector.memset(Lf[D:D + 2, :], 1.0)
            nc.sync.dma_start(out=RXf, in_=RX_d[:])
            nc.sync.dma_start(out=RYf, in_=RY_d[:])
            nc.sync.dma_start(out=EYEf, in_=EYE_d[:])
            if not bf16x2:
                nc.vector.tensor_scalar_mul(Lf[0:D, :], RXf[0:D, :], -2.0)

            Lr = const.tile([KA, N], mmdt)
            RXr = const.tile([KA, N], mmdt)
            RYr = const.tile([KA, N], mmdt)
            EYEr = const.tile([P, P], mmdt)
            nc.vector.tensor_copy(Lr, Lf)
            nc.vector.tensor_copy(RXr, RXf)
            nc.vector.tensor_copy(RYr, RYf)
            nc.vector.tensor_copy(EYEr, EYEf)
            if bf16x2:
                L2f = const.tile([K2, N], f32)
                RX2f = const.tile([K2, N], f32)
                RY2f = const.tile([K2, N], f32)
                nc.sync.dma_start(out=L2f, in_=L2_d[:])
                nc.sync.dma_start(out=RX2f, in_=RX2_d[:])
                nc.sync.dma_start(out=RY2f, in_=RY2_d[:])
                L2r = const.tile([K2, N], mmdt)
                RX2r = const.tile([K2, N], mmdt)
                RY2r = const.tile([K2, N], mmdt)
                nc.vector.tensor_copy(L2r, L2f)
                nc.vector.tensor_copy(RX2r, RX2f)
                nc.vector.tensor_copy(RY2r, RY2f)

            mins_x = outs.tile([P, n_tiles * NB], f32)
            mins_y = outs.tile([P, n_tiles * NB], f32)

            for _ in range(repeat):
                for bi in range(NB):
                    lhs = Lr[:, bi * P:(bi + 1) * P]
                    if bf16x2:
                        mats = ((RXr, RX2r, mins_x, True), (RYr, RY2r, mins_y, False))
                    else:
                        mats = ((RXr, None, mins_x, True), (RYr, None, mins_y, False))
                    for R, R2, mins, is_xx in mats:
                        for h in range(n_tiles):
                            p = psum.tile([P, red_w], f32, tag="p")
                            for c in range(n_ch):
                                col0 = h * red_w + c * mm_w
                                diag_here = is_xx and col0 <= bi * P < col0 + mm_w
                                sl = p[:, c * mm_w:(c + 1) * mm_w]
                                last = not (diag_here or bf16x2)
                                if skip_mm and not (bi == 0 and c == 0):
                                    continue
                                nc.tensor.matmul(
                                    sl, lhs, R[:, col0:col0 + mm_w],
                                    start=True, stop=last)
                                if bf16x2:
                                    nc.tensor.matmul(
                                        sl, L2r[:, bi * P:(bi + 1) * P],
                                        R2[:, col0:col0 + mm_w],
                                        start=False, stop=not diag_here,
                                        skip_group_check=True)
                                if diag_here:
                                    off = c * mm_w + (bi * P - col0)
                                    nc.tensor.matmul(
                                        p[:, off:off + P], EYEr[:], EYEr[:],
                                        start=False, stop=True,
                                        skip_group_check=True)
                            if skip_reduce:
                                nc.vector.tensor_reduce(
                                    out=mins[:, n_tiles * bi + h:n_tiles * bi + h + 1],
                                    in_=p[:, 0:2], axis=mybir.AxisListType.X,
                                    op=mybir.AluOpType.min)
                            else:
                                nc.vector.tensor_reduce(
                                    out=mins[:, n_tiles * bi + h:n_tiles * bi + h + 1],
                                    in_=p[:], axis=mybir.AxisListType.X,
                                    op=mybir.AluOpType.min)

            nc.sync.dma_start(out=MX_d[:], in_=mins_x)
            nc.sync.dma_start(out=MY_d[:], in_=mins_y)

    nc.finalize()
    return nc




def _build_raw(repeat=1, use_fp32=False):
    """Raw-bacc variant: no Tile framework, sems ride on compute instructions.

    Cuts the 128 standalone per-iteration EventSemaphore instructions the Tile
    scheduler emits (2 per PSUM tile). Handshake per psum tile t (0..64R-1):
      PE: first matmul of tile t waits dve_sem >= t (psum free), last matmul
          then_inc(pe_sem).  DVE: reduce t waits pe_sem >= t+1, then_inc(dve_sem).
    """
    from contextlib import ExitStack
    f32 = mybir.dt.float32
    f32r = mybir.dt.float32r
    nc = bacc.Bacc(None, target_bir_lowering=False)
    RX_d = nc.dram_tensor("RX", [K, N], f32, kind="ExternalInput")
    RY_d = nc.dram_tensor("RY", [K, N], f32, kind="ExternalInput")
    EYE_d = nc.dram_tensor("EYE", [P, P], f32, kind="ExternalInput")
    MX_d = nc.dram_tensor("MX", [P, NB], f32, kind="ExternalOutput")
    MY_d = nc.dram_tensor("MY", [P, NB], f32, kind="ExternalOutput")

    n_tiles_total = 2 * NB * repeat

    with ExitStack() as ctx:
        RXf = ctx.enter_context(nc.sbuf_tensor([K, N], f32))
        RYf = ctx.enter_context(nc.sbuf_tensor([K, N], f32))
        Lf = ctx.enter_context(nc.sbuf_tensor([K, N], f32))
        EYEf = ctx.enter_context(nc.sbuf_tensor([P, P], f32))
        Lr = ctx.enter_context(nc.sbuf_tensor([K, N], f32r))
        RXr = ctx.enter_context(nc.sbuf_tensor([K, N], f32r))
        RYr = ctx.enter_context(nc.sbuf_tensor([K, N], f32r))
        EYEr = ctx.enter_context(nc.sbuf_tensor([P, P], mybir.dt.bfloat16))
        mins_x = ctx.enter_context(nc.sbuf_tensor([P, NB], f32))
        mins_y = ctx.enter_context(nc.sbuf_tensor([P, NB], f32))
        psum = ctx.enter_context(nc.psum_tensor([P, N], f32))
        dma_sem = ctx.enter_context(nc.semaphore())
        conv_sem = ctx.enter_context(nc.semaphore())
        pe_sem = ctx.enter_context(nc.semaphore())
        dve_sem = ctx.enter_context(nc.semaphore())
        block = ctx.enter_context(nc.Block())

        @block.sync
        def _(sync):
            sync.dma_start(out=RXf[:], in_=RX_d[:]).then_inc(dma_sem, 16)
            sync.dma_start(out=RYf[:], in_=RY_d[:]).then_inc(dma_sem, 16)
            sync.dma_start(out=EYEf[:], in_=EYE_d[:]).then_inc(dma_sem, 16)
            sync.wait_ge(dve_sem, n_tiles_total)
            sync.dma_start(out=MX_d[:], in_=mins_x[:]).then_inc(dma_sem, 16)
            sync.dma_start(out=MY_d[:], in_=mins_y[:]).then_inc(dma_sem, 16)

        if use_fp32:
            Lr, RXr, RYr, EYEr = Lf, RXf, RYf, EYEf

        @block.vector
        def _(vector):
            vector.wait_ge(dma_sem, 48)
            vector.memset(Lf[D:D + 2, :], 1.0)
            mul = nc.vector.tensor_scalar_mul(Lf[0:D, :], RXf[0:D, :], -2.0)
            if use_fp32:
                mul.then_inc(conv_sem, 1)
            else:
                nc.vector.tensor_copy(Lr[:], Lf[:])
                nc.vector.tensor_copy(RXr[:], RXf[:])
                nc.vector.tensor_copy(RYr[:], RYf[:])
                nc.vector.tensor_copy(EYEr[:], EYEf[:]).then_inc(conv_sem, 1)
            t = 0
            for _ in range(repeat):
                for bi in range(NB):
                    for mins in (mins_x, mins_y):
                        vector.wait_ge(pe_sem, t + 1)
                        nc.vector.tensor_reduce(
                            out=mins[:, bi:bi + 1], in_=psum[:],
                            axis=mybir.AxisListType.X,
                            op=mybir.AluOpType.min).then_inc(dve_sem, 1)
                        t += 1

        @block.tensor
        def _(tensor):
            tensor.wait_ge(conv_sem, 1)
            t = 0
            for _ in range(repeat):
                for bi in range(NB):
                    lhs = Lr[:, bi * P:(bi + 1) * P]
                    for R, is_xx in ((RXr, True), (RYr, False)):
                        if t > 0:
                            tensor.wait_ge(dve_sem, t)
                        for c in range(8):
                            col0 = c * 512
                            diag_here = is_xx and col0 <= bi * P < col0 + 512
                            mm = nc.tensor.matmul(
                                psum[:, col0:col0 + 512],
                                lhs, R[:, col0:col0 + 512],
                                start=True, stop=not diag_here)
                            if diag_here:
                                off = bi * P
                                mm = nc.tensor.matmul(
                                    psum[:, off:off + P], EYEr[:], EYEr[:],
                                    start=False, stop=True,
                                    skip_group_check=True)
                            if c == 7:
                                mm.then_inc(pe_sem, 1)
                        t += 1

    nc.finalize()
    return nc


def _get_nc(repeat=1, mmdt_name=None, mm_w=None, red_w=None):
    key = (repeat, mmdt_name or MM_DTYPE, mm_w or MM_W, red_w or RED_W)
    if key not in _cache:
        _cache[key] = _build(repeat, mmdt_name, mm_w, red_w)
    return _cache[key]


def _get_raw_nc(repeat=1):
    key = ("raw", repeat)
    if key not in _cache:
        _cache[key] = _build_raw(repeat)
    return _cache[key]


def _hi_round(v):
    # hi part must be exactly representable in the matmul dtype
    if MM_DTYPE == "float16":
        return v.astype(np.float32).astype(np.float16).astype(np.float64)
    return v.astype(np.float32).astype(ml_dtypes.bfloat16).astype(np.float64)


def _bf16(v):
    return v.astype(np.float32).astype(ml_dtypes.bfloat16).astype(np.float64)


def _prep_maps(X, Y):
    X = np.asarray(X, dtype=np.float32)
    Y = np.asarray(Y, dtype=np.float32)
    eye = (np.eye(P) * SQRT_BIG).astype(np.float32)
    in_maps = []
    x2_all = []
    for b in range(B):
        Xb = X[b].astype(np.float64)
        Yb = Y[b].astype(np.float64)
        x2 = (Xb * Xb).sum(1)
        y2 = (Yb * Yb).sum(1)
        ones = np.ones((1, N), dtype=np.float64)
        if MM_DTYPE == "bf16x2":
            Xh = _bf16(Xb); Xl = Xb - Xh
            Yh = _bf16(Yb); Yl = Yb - Yh
            x2h = _bf16(x2); y2h = _bf16(y2)
            L = np.concatenate([-2.0 * Xh.T, ones, ones], 0).astype(np.float32)
            RX = np.concatenate([Xh.T, x2h[None], (x2 - x2h)[None]], 0).astype(np.float32)
            RY = np.concatenate([Yh.T, y2h[None], (y2 - y2h)[None]], 0).astype(np.float32)
            L2 = np.concatenate([-2.0 * Xh.T, -2.0 * Xl.T], 0).astype(np.float32)
            RX2 = np.concatenate([Xl.T, Xh.T], 0).astype(np.float32)
            RY2 = np.concatenate([Yl.T, Yh.T], 0).astype(np.float32)
            in_maps.append({"L": L, "RX": RX, "RY": RY,
                            "L2": L2, "RX2": RX2, "RY2": RY2, "EYE": eye})
        else:
            x2h = _hi_round(x2)
            y2h = _hi_round(y2)
            RX = np.concatenate([Xb.T, x2h[None], (x2 - x2h)[None]], 0).astype(np.float32)
            RY = np.concatenate([Yb.T, y2h[None], (y2 - y2h)[None]], 0).astype(np.float32)
            in_maps.append({"RX": RX, "RY": RY, "EYE": eye})
        x2_all.append(x2)
    return in_maps, x2_all


def _postprocess(results, x2_all):
    out = np.zeros(B, dtype=np.float64)
    for b in range(B):
        mx = results[b]["MX"].astype(np.float64)  # [P, n_tiles*NB]
        my = results[b]["MY"].astype(np.float64)
        nt = mx.shape[1] // NB
        # [p, bi, h] -> min over tiles -> [p, bi] -> row i = bi*P + p
        d2x = mx.reshape(P, NB, nt).min(2).T.reshape(-1)
        d2y = my.reshape(P, NB, nt).min(2).T.reshape(-1)
        d2x = d2x + x2_all[b]
        d2y = d2y + x2_all[b]
        d2x = np.maximum(d2x, EPS)
        d2y = np.maximum(d2y, EPS)
        out[b] = 0.5 * np.mean(np.log(d2x) - np.log(d2y))
    return out.astype(np.float32)


def _run_with_retry(nc, in_maps):
    for attempt in range(2):
        try:
            return run_bass_kernel_spmd(nc, in_maps, core_ids=list(range(B))).results
        except Exception:
            time.sleep(3)
    # last resort: one batch at a time, skipping wedged cores
    results = [None] * B
    for b in range(B):
        for c in range(8):
            core = (b + c) % 8
            try:
                results[b] = run_bass_kernel_spmd(
                    nc, [in_maps[b]], core_ids=[core]).results[0]
                break
            except Exception:
                continue
        if results[b] is None:
            raise RuntimeError("all cores failed")
    return results


def kernel(X, Y):
    in_maps, x2_all = _prep_maps(X, Y)
    try:
        results = _run_with_retry(_get_raw_nc(1), in_maps)
    except Exception:
        # fall back to the Tile-framework build
        results = _run_with_retry(_get_nc(repeat=1), in_maps)
    return _postprocess(results, x2_all)


# Pre-build the default program at import time so the first kernel() call
# doesn't pay Bass graph construction; guarded so import can never fail.
try:
    _get_raw_nc(1)
except Exception:
    pass


if __name__ == "__main__":
    rng = np.random.default_rng(0)
    X = rng.standard_normal((B, N, D)).astype(np.float32)
    Y = rng.standard_normal((B, N, D)).astype(np.float32)
    print(kernel(X, Y))



# revision 3
# speedup vs baseline: 153.8740x; 153.8740x over previous
"""Trainium2 Bass kernel for nn_ExactDivergenceModel (retrieval_knn).

Backend reality (measured): each run of a program through this axon stack
costs ~30-50us PER STATIC NEFF INSTRUCTION in dispatch overhead, while
hardware-loop iterations are nearly free (just engine compute). The
straightforward kernel (~690 static instructions) therefore costs ~20ms;
this one restructures the identical math into per-engine hardware Fori
loops over the 32 row-blocks -> ~55 static instructions.

Math per batch b (one NeuronCore each):
  v_xx[i,j] = x2[j] - 2<X_i,X_j>,  v_yx[i,j] = y2[j] - 2<X_i,Y_j>
  d2min[i]  = min_j v[i,j] + x2[i]        (XX: diag masked with +2^30)
  out[b]    = 0.5 * mean_i(log d2min_xx[i] - log d2min_yx[i])

Per round q = bi (HW loop on every engine):
  DVE:  stage L[:, bi*128:+128] into the fixed Lstage tile (matmul weights
        APs must have static offsets -> vary the DATA, not the AP), then
        per gen one tensor_reduce(min) over the whole [128,4096] PSUM into
        MX/MY[:, bi] (register-offset slot).
  PE :  XX gen: 8 static 512-wide f32r matmuls psum[c] = Lstage.T @ RX[c],
        + eye matmul accumulating 2^30*I at the diag block (register
        offset out), then YX gen likewise from RY.

Sync: monotone counting semaphores with REGISTER-VALUED wait thresholds
(engine registers bumped per iteration). Decrement-based credit schemes
hang this runtime; register-valued waits are verified to work on it.

Host adds x2[i] (the augmented-matmul rows carry x2[j] split hi/lo so the
f32r operand rounding cannot corrupt it), clamps, and takes logs in fp64.
"""
import sys, time
sys.path.insert(0, '/opt/trn_rl_repo')

import numpy as np
from contextlib import ExitStack

import concourse.bass as bass
from concourse.bass import ds
from concourse import bacc, mybir
from concourse.bass_utils import run_bass_kernel_spmd

B, N, D = 8, 4096, 64
P = 128
NB = N // P             # 32 rounds
K = D + 2               # 66
NCH = N // 512          # 8 chunks per gen
EPS = 1e-12
SQRT_BIG = 32768.0      # 2^15; adds 2^30 on the XX diagonal

_cache = {}

f32 = mybir.dt.float32
f32r = mybir.dt.float32r


def _build(repeat=1, nb=NB):
    nc = bacc.Bacc(None, target_bir_lowering=False)
    L_d = nc.dram_tensor("L", [K, N], f32r, kind="ExternalInput")
    RX_d = nc.dram_tensor("RX", [K, N], f32r, kind="ExternalInput")
    RY_d = nc.dram_tensor("RY", [K, N], f32r, kind="ExternalInput")
    EYE_d = nc.dram_tensor("EYE", [P, P], f32r, kind="ExternalInput")
    MX_d = nc.dram_tensor("MX", [P, NB], f32, kind="ExternalOutput")
    MY_d = nc.dram_tensor("MY", [P, NB], f32, kind="ExternalOutput")

    n_rounds = nb * repeat
    n_gens = 2 * n_rounds

    with ExitStack() as ctx:
        Lr = ctx.enter_context(nc.sbuf_tensor([K, N], f32r))
        RXr = ctx.enter_context(nc.sbuf_tensor([K, N], f32r))
        RYr = ctx.enter_context(nc.sbuf_tensor([K, N], f32r))
        EYEr = ctx.enter_context(nc.sbuf_tensor([P, P], f32r))
        Lstage = ctx.enter_context(nc.sbuf_tensor([K, P], f32r))
        MX = ctx.enter_context(nc.sbuf_tensor([P, NB], f32))
        MY = ctx.enter_context(nc.sbuf_tensor([P, NB], f32))
        psum = ctx.enter_context(nc.psum_tensor([P, N], f32))
        dma_sem = ctx.enter_context(nc.semaphore())
        stage_done = ctx.enter_context(nc.semaphore())  # staged rounds
        pe_done = ctx.enter_context(nc.semaphore())     # completed PE gens
        dve_done = ctx.enter_context(nc.semaphore())    # completed reduces
        block = ctx.enter_context(nc.Block())

        @block.sync
        def _(sync):
            sync.dma_start(out=Lr[:], in_=L_d[:]).then_inc(dma_sem, 16)
            sync.dma_start(out=RXr[:], in_=RX_d[:]).then_inc(dma_sem, 16)
            sync.dma_start(out=RYr[:], in_=RY_d[:]).then_inc(dma_sem, 16)
            sync.dma_start(out=EYEr[:], in_=EYE_d[:]).then_inc(dma_sem, 16)
            sync.wait_ge(dve_done, n_gens)
            sync.dma_start(out=MX_d[:], in_=MX[:]).then_inc(dma_sem, 16)
            sync.dma_start(out=MY_d[:], in_=MY[:]).then_inc(dma_sem, 16)

        @block.vector
        def _(vector):
            vector.wait_ge(dma_sem, 64)
            ro = vector.alloc_register("ro")        # stage src offset: 128q
            svo = nc.snap(ro, donate=True, min_val=0, max_val=N - P)
            rsl = vector.alloc_register("rsl")      # mins slot: q
            svsl = nc.snap(rsl, donate=True, min_val=0, max_val=NB - 1)
            rsp = vector.alloc_register("rsp")      # pe gate stage: 2q
            svsp = nc.snap(rsp, donate=True, min_val=0, max_val=n_gens)
            rrx = vector.alloc_register("rrx")      # pe gate reduce XX: 2q+1
            svrx = nc.snap(rrx, donate=True, min_val=1, max_val=n_gens)
            rry = vector.alloc_register("rry")      # pe gate reduce YX: 2q+2
            svry = nc.snap(rry, donate=True, min_val=2, max_val=n_gens)
            vector.reg_mov(rsp, 0)
            vector.reg_mov(rrx, 1)
            vector.reg_mov(rry, 2)
            with vector.Fori(0, repeat):
                vector.reg_mov(ro, 0)
                vector.reg_mov(rsl, 0)
                with vector.Fori(0, nb):
                    # Lstage free once YX matmuls of round q-1 completed
                    vector.wait_ge(pe_done, svsp)
                    nc.vector.tensor_copy(
                        Lstage[:], Lr[:, ds(svo, P)]).then_inc(stage_done, 1)
                    vector.wait_ge(pe_done, svrx)
                    nc.vector.tensor_reduce(
                        out=MX[:, ds(svsl, 1)], in_=psum[:],
                        axis=mybir.AxisListType.X,
                        op=mybir.AluOpType.min).then_inc(dve_done, 1)
                    vector.wait_ge(pe_done, svry)
                    nc.vector.tensor_reduce(
                        out=MY[:, ds(svsl, 1)], in_=psum[:],
                        axis=mybir.AxisListType.X,
                        op=mybir.AluOpType.min).then_inc(dve_done, 1)
                    vector.reg_add(ro, ro, P)
                    vector.reg_add(rsl, rsl, 1)
                    vector.reg_add(rsp, rsp, 2)
                    vector.reg_add(rrx, rrx, 2)
                    vector.reg_add(rry, rry, 2)

        @block.tensor
        def _(tensor):
            tensor.wait_ge(dma_sem, 64)
            rd = tensor.alloc_register("rd")        # diag offset: 128q
            svd = nc.snap(rd, donate=True, min_val=0, max_val=N - P)
            rsd = tensor.alloc_register("rsd")      # stage gate: q+1
            svsd = nc.snap(rsd, donate=True, min_val=1, max_val=n_rounds)
            rxx = tensor.alloc_register("rxx")      # dve gate XX: 2q
            svxx = nc.snap(rxx, donate=True, min_val=0, max_val=n_gens)
            ryx = tensor.alloc_register("ryx")      # dve gate YX: 2q+1
            svyx = nc.snap(ryx, donate=True, min_val=1, max_val=n_gens)
            tensor.reg_mov(rsd, 1)
            tensor.reg_mov(rxx, 0)
            tensor.reg_mov(ryx, 1)
            with tensor.Fori(0, repeat):
                tensor.reg_mov(rd, 0)
                with tensor.Fori(0, nb):
                    # ---- XX gen ----
                    tensor.wait_ge(stage_done, svsd)
                    tensor.wait_ge(dve_done, svxx)
                    for c in range(NCH):
                        nc.tensor.matmul(
                            psum[:, c * 512:(c + 1) * 512], Lstage[:],
                            RXr[:, c * 512:(c + 1) * 512],
                            start=True, stop=True)
                    # diag mask: += 2^30 * I at cols [bi*128, +128)
                    nc.tensor.matmul(
                        psum[:, ds(svd, P)], EYEr[:], EYEr[:],
                        start=False, stop=True,
                        skip_group_check=True).then_inc(pe_done, 1)
                    # ---- YX gen ----
                    tensor.wait_ge(dve_done, svyx)
                    for c in range(NCH - 1):
                        nc.tensor.matmul(
                            psum[:, c * 512:(c + 1) * 512], Lstage[:],
                            RYr[:, c * 512:(c + 1) * 512],
                            start=True, stop=True)
                    c = NCH - 1
                    nc.tensor.matmul(
                        psum[:, c * 512:(c + 1) * 512], Lstage[:],
                        RYr[:, c * 512:(c + 1) * 512],
                        start=True, stop=True).then_inc(pe_done, 1)
                    tensor.reg_add(rd, rd, P)
                    tensor.reg_add(rsd, rsd, 1)
                    tensor.reg_add(rxx, rxx, 2)
                    tensor.reg_add(ryx, ryx, 2)

    nc.finalize()
    return nc


def _get_nc(repeat=1):
    if repeat not in _cache:
        _cache[repeat] = _build(repeat)
    return _cache[repeat]


def _prep_maps(X, Y):
    import ml_dtypes
    X = np.asarray(X, dtype=np.float32)
    Y = np.asarray(Y, dtype=np.float32)
    eye = (np.eye(P) * SQRT_BIG).astype(np.float32)
    in_maps = []
    x2_all = []
    for b in range(B):
        Xb = X[b].astype(np.float64)
        Yb = Y[b].astype(np.float64)
        x2 = (Xb * Xb).sum(1)
        y2 = (Yb * Yb).sum(1)
        ones = np.ones((1, N), dtype=np.float64)
        # hi part bf16-representable so it survives f32r operand rounding
        x2h = x2.astype(np.float32).astype(ml_dtypes.bfloat16).astype(np.float64)
        y2h = y2.astype(np.float32).astype(ml_dtypes.bfloat16).astype(np.float64)
        L = np.concatenate([-2.0 * Xb.T, ones, ones], 0).astype(np.float32)
        RX = np.concatenate([Xb.T, x2h[None], (x2 - x2h)[None]], 0).astype(np.float32)
        RY = np.concatenate([Yb.T, y2h[None], (y2 - y2h)[None]], 0).astype(np.float32)
        in_maps.append({"L": L, "RX": RX, "RY": RY, "EYE": eye})
        x2_all.append(x2)
    return in_maps, x2_all


def _postprocess(results, x2_all):
    out = np.zeros(B, dtype=np.float64)
    for b in range(B):
        mx = results[b]["MX"].astype(np.float64).T.reshape(-1)  # row i = bi*P+p
        my = results[b]["MY"].astype(np.float64).T.reshape(-1)
        x2 = x2_all[b]
        d2x = np.maximum(mx + x2, EPS)
        d2y = np.maximum(my + x2, EPS)
        out[b] = 0.5 * np.mean(np.log(d2x) - np.log(d2y))
    return out.astype(np.float32)


def _run_with_retry(nc, in_maps):
    last = None
    for attempt in range(3):
        try:
            return run_bass_kernel_spmd(nc, in_maps, core_ids=list(range(B))).results
        except Exception as ex:
            last = ex
            time.sleep(3)
    raise last


def kernel(X, Y):
    in_maps, x2_all = _prep_maps(X, Y)
    results = _run_with_retry(_get_nc(1), in_maps)
    return _postprocess(results, x2_all)


# Pre-build the default program at import time; guarded so import never fails.
try:
    _get_nc(1)
except Exception:
    pass


if __name__ == "__main__":
    rng = np.random.default_rng(0)
    X = rng.standard_normal((B, N, D)).astype(np.float32)
    Y = rng.standard_normal((B, N, D)).astype(np.float32)
    t0 = time.time()
    got = kernel(X, Y)
    print("kernel:", got, f"({time.time()-t0:.1f}s)")
    exp = np.zeros(B)
    for b in range(B):
        Xb, Yb = X[b].astype(np.float64), Y[b].astype(np.float64)
        x2 = (Xb**2).sum(1); y2 = (Yb**2).sum(1)
        vxx = x2[None] - 2*(Xb@Xb.T); np.fill_diagonal(vxx, 1e9)
        vyx = y2[None] - 2*(Xb@Yb.T)
        dx = np.maximum(vxx.min(1) + x2, EPS)
        dy = np.maximum(vyx.min(1) + x2, EPS)
        exp[b] = 0.5*np.mean(np.log(dx) - np.log(dy))
    print("exact ref:", exp)
    print("rel err:", np.linalg.norm(got-exp)/np.linalg.norm(exp))
